# revision 47
# baseline (speedup 1.0000x reference)
"""Trainium2 Bass kernel for nn_MultiHeadAttention_36223754174786.

Fused transformer block: QKV projection -> 16-head attention (naive, full
[S,S] scores) -> LayerNorm -> FeedForward (relu MLP) with residual.
B=2, S=2048, D=1024, H=16, DK=64, FF_HIDDEN=2048.

Sharding: data-parallel over tokens across 8 NeuronCores.  Core c handles 512
query tokens of batch b=c//4.  K/V projections for the full batch are
recomputed on each core: cross-core exchange is infeasible here (remote_dma
deadlocks the Tile scheduler's sim; collective_compute is priced 15us+40GB/s
in the cost model), so no cross-core communication at all.

Numerics -- compensated fp8 DoubleRow (the workhorse):
  A plain fp8e4 operand costs ~1.2e-2 on the rel-err metric (the metric is
  brutally sensitive: attention output pre-LN has std ~0.02, so LayerNorm
  amplifies any attn-path error ~50x).  Instead, every host-prepared matmul
  (Q/K/V projections, FFN W1/W2) runs THREE fp8-DR products per chunk-pair:
    x@w ~= x8@w8 + xr8@w8 + x8@wr8,   x8=f8(SX*x), xr8=f8(SX*x - x8),
                                      w8=f8(SW*w), wr8=f8(SW*w - w8)
  with SX=4, SW=256 chosen so mains AND residuals sit in fp8's normal range.
  Residual-compensation leaves only the xr*wr cross term (~0.1%, measured
  1.3e-3 end-to-end).  Cost: 3 DR mms (0.5 c/r each) per pair = 0.75x the
  fp16 cost; fp16 elsewhere (scores, attnV, exp) -- fp16 is priced like bf16
  (1.0 c/r) but has 8x finer mantissa, so it's free accuracy.
  Every activation in the fused chain then carries a uniform SC=SX*SW=1024
  scale: exp() folds it away via its scale operand (1/(SC^2*sqrt(dk))),
  LayerNorm is scale-invariant BUT its epsilon must be scaled by SC^2
  (1e-5 at native scale is a 2.4% variance perturbation -- dropping it
  costs 2e-2 on the metric!), and the FFN residual-add unscales by 1/SC.

Attention layout (vs the 353us baseline):
  scores are produced TRANSPOSED, sT[keys,q] (16 psum chunks per head),
  exp'd straight out of PSUM on ScalarE into fp16 SBUF tiles e[keys,2*512q].
  attnV computes token-major out[q, dk] per head with lhsT=e (128-col
  slices) and rhs=[V_h | 1] token-major [keys, 65]: moving dim 65 instead
  of 512 -- and row 64 accumulates the softmax denominator, the output
  lands token-major, so NO PE transposes or per-q reciprocal loops.  The 4
  q-chunk accumulators share one PSUM bank (start once, stop once).

PE rows/core: Qp 24.6k + Kp 98.3k + Vp 98.3k + scores 131k + attnV 66.5k +
W1 49.1k + W2 49.1k + LN-tr 8.2k ~= 525k ~= 226us busy; ScalarE exp ~137us.
Timeline-sim total: 281.2us (was 353.5us baseline); rel err 1.45e-3.
The xk/wk prefetch is split across two p3 token-groups: the attention phase
is ScalarE-bound (137us of exp), so its start is gated by these DMA arrivals
competing with the xv stream for bandwidth.
Scores cannot go compensated-fp8: contraction is only 64 deep, so DoubleRow
gives no depth advantage and 3 products cost 1.5x the single fp16 matmul.
"""

import numpy as np

import concourse.bass as bass
import concourse.tile as tile
from concourse import bacc, mybir
from concourse.bass_utils import run_bass_kernel_spmd
from concourse.masks import make_identity

F32 = mybir.dt.float32
F16 = mybir.dt.float16
F8 = mybir.dt.float8e4
DR = mybir.MatmulPerfMode.DoubleRow
AF = mybir.ActivationFunctionType
OP = mybir.AluOpType

B, S, D, H = 2, 2048, 1024, 16
DK = D // H          # 64
FF = 2048
P = 128
T = 512              # query tokens per core
N_CORES = 8
KC = S // P          # 16 key chunks
QS = T // P          # 4 query sub-tiles
DCH = D // P         # 8 chunks of the model dim
DPR = DCH // 2       # 4 fp8 pair-chunks of the model dim
FPR = FF // 256      # 8 fp8 pair-chunks of the ffn hidden dim
SX = 4.0             # fp8 scale, activation side
SW = 256.0           # fp8 scale, weight side
SC = SX * SW         # scale carried by every projected activation
EXP_SCALE = 1.0 / (SC * SC * 8.0)   # un-scales q*k and folds 1/sqrt(dk)
LN_EPS = 1e-5 * SC * SC             # attn3 carries SC => var carries SC^2


def _bcast_ap(ap):
    """Partition-broadcast a 1-D DRAM vector to [128, n] for DMA."""
    return bass.AP(tensor=ap.tensor, offset=ap.offset, ap=[[0, P]] + list(ap.ap))


def _bcast_free(ap, n):
    """Append a stride-0 free dim of size n (broadcast-read for DVE ops)."""
    return bass.AP(tensor=ap.tensor, offset=ap.offset,
                   ap=list(ap.ap) + [[0, n]])


def build_program(ln_affine=True, bq_zero=False, bk_zero=False, bv_zero=False,
                  b1_zero=False, b2_zero=False):
    nc = bacc.Bacc("TRN2", target_bir_lowering=False, debug=False,
                   num_devices=N_CORES)

    def mm(out_ap, lhsT, rhs, start, stop, perf_mode=None):
        nc.tensor.matmul(out_ap, lhsT, rhs, start=start, stop=stop,
                         perf_mode=perf_mode)

    def dram_pair(name, shape):
        """main + residual fp8 DRAM inputs."""
        return (nc.dram_tensor(name, shape, F8, kind="ExternalInput"),
                nc.dram_tensor(name + "r", shape, F8, kind="ExternalInput"))

    xq8, xqr8 = dram_pair("xq8", [D, T])
    wq8, wqr8 = dram_pair("wq8", [D, D])
    xk8, xkr8 = dram_pair("xk8", [D, S])
    wk8, wkr8 = dram_pair("wk8", [D, D])
    xv8, xvr8 = dram_pair("xv8", [D, S])
    wv8, wvr8 = dram_pair("wv8", [D, D])
    w18, w1r8 = dram_pair("w18", [D, FF])
    w28, w2r8 = dram_pair("w28", [FF, D])
    bq = nc.dram_tensor("bq", [D], F32, kind="ExternalInput")
    bk = nc.dram_tensor("bk", [D], F32, kind="ExternalInput")
    bv = nc.dram_tensor("bv", [D], F32, kind="ExternalInput")
    b1 = nc.dram_tensor("b1", [FF], F32, kind="ExternalInput")
    b2 = nc.dram_tensor("b2", [D], F32, kind="ExternalInput")
    ln_g = nc.dram_tensor("ln_g", [D], F32, kind="ExternalInput")
    ln_b = nc.dram_tensor("ln_b", [D], F32, kind="ExternalInput")
    out = nc.dram_tensor("out", [T, D], F32, kind="ExternalOutput")

    def load_pairs(pool, tag, main, resid, cols, col0=0, npairs=DPR,
                   eng=None, pair0=0):
        """fp8 pair-tiles [P, 2, cols]: slot i of pair k holds DRAM rows
        256k+128i.., so [:, :, slice] is the DR [Ki, 2, dim] layout.
        Returns (mains, resids), each a list of npairs tiles."""
        eng = eng or nc.sync
        ms, rs = [], []
        for k in range(pair0, pair0 + npairs):
            for lst, src, nm in ((ms, main, tag), (rs, resid, tag + "r")):
                t_ = pool.tile([P, 2, cols], F8, tag=f"{nm}{k}",
                               name=f"{nm}{k}")
                eng.dma_start(
                    t_,
                    src[256 * k:256 * (k + 1), col0:col0 + cols].rearrange(
                        "(i p) c -> p i c", i=2))
                lst.append(t_)
        return ms, rs

    def emit_comp(ps, wm, wr, xm, xr, wsl, xsl, start):
        """One chunk-pair of the compensated product into psum `ps`:
        w8.T@x8 + w8.T@xr8 + wr8.T@x8, each a DoubleRow matmul."""
        mm(ps, wm[:, :, wsl], xm[:, :, xsl], start=start, stop=False,
           perf_mode=DR)
        mm(ps, wm[:, :, wsl], xr[:, :, xsl], start=False, stop=False,
           perf_mode=DR)
        return lambda stop: mm(ps, wr[:, :, wsl], xm[:, :, xsl], start=False,
                               stop=stop, perf_mode=DR)

    def emit_comp_seq(ps, wms, wrs, xms, xrs, wsl, xsl):
        """Full contraction (DPR pairs x 3 products) into psum `ps`."""
        for k in range(DPR):
            last = emit_comp(ps, wms[k], wrs[k], xms[k], xrs[k], wsl, xsl,
                             start=(k == 0))
            last(stop=(k == DPR - 1))

    def emit_p1(qT, bq_col, p1w, acc):
        xm, xr = load_pairs(p1w, "xq", xq8, xqr8, T)
        wm, wr = load_pairs(p1w, "wq", wq8, wqr8, D)
        for m in range(DCH):
            ps = acc.tile([P, 512], F32, tag="acc", name="acc")
            emit_comp_seq(ps, wm, wr, xm, xr,
                          slice(m * P, (m + 1) * P), slice(None))
            if bq_zero:
                nc.vector.tensor_copy(qT[m], ps)
            else:
                nc.vector.tensor_scalar_add(qT[m], ps, bq_col[:, m:m + 1])

    def emit_p3(v_sb, bv_b, ones_t, p3w, p3x, acc, prefetch=None,
                prefetch2=None):
        wm, wr = load_pairs(p3w, "wv", wv8, wvr8, D)
        for tg in range(KC // 4):
            xm, xr = load_pairs(p3x, "xv", xv8, xvr8, 512, col0=tg * 512)
            if tg in (1, 2) and prefetch is not None:
                nc._xkwk = prefetch(tg - 1)
            if tg == 2 and prefetch2 is not None:
                prefetch2()
            for t in range(tg * 4, tg * 4 + 4):
                nc.vector.tensor_copy(v_sb[t][:, :, DK:DK + 1], ones_t)
                for dch in range(2):
                    ps = acc.tile([P, 512], F32, tag="acc", name="acc")
                    # token-major: lhsT = x pairs, rhs = w pairs
                    for k in range(DPR):
                        tsl = slice((t - tg * 4) * P, (t - tg * 4 + 1) * P)
                        csl = slice(dch * 512, (dch + 1) * 512)
                        mm(ps, xm[k][:, :, tsl], wm[k][:, :, csl],
                           start=(k == 0), stop=False, perf_mode=DR)
                        mm(ps, xr[k][:, :, tsl], wm[k][:, :, csl],
                           start=False, stop=False, perf_mode=DR)
                        mm(ps, xm[k][:, :, tsl], wr[k][:, :, csl],
                           start=False, stop=(k == DPR - 1), perf_mode=DR)
                    if bv_zero:
                        nc.vector.tensor_copy(
                            v_sb[t][:, dch * 8:(dch + 1) * 8, 0:DK],
                            ps[:].rearrange("p (h d) -> p h d", h=8))
                    else:
                        nc.vector.tensor_tensor(
                            v_sb[t][:, dch * 8:(dch + 1) * 8, 0:DK],
                            ps[:].rearrange("p (h d) -> p h d", h=8),
                            bv_b[:, dch * 512:(dch + 1) * 512].rearrange(
                                "p (h d) -> p h d", h=8),
                            OP.add)

    def load_xk_wk(p2w, part):
        """two parts (p3 tg 1 and 2) to smooth DMA contention with xv"""
        if part == 0:
            nc._xkh0 = load_pairs(p2w, "xk", xk8, xkr8, S, npairs=2)
            return None
        xm2, xr2 = load_pairs(p2w, "xkB", xk8, xkr8, S, npairs=2, pair0=2)
        wm, wr = load_pairs(p2w, "wk", wk8, wkr8, D)
        xm, xr = nc._xkh0
        return xm + xm2, xr + xr2, wm, wr

    def emit_kp(p, xk_t, aK, acc):
        xm, xr, wm, wr = xk_t
        kp = aK.tile([P, S], F16, tag="kp", name="kp")
        for nch in range(S // 512):
            ps = acc.tile([P, 512], F32, tag="acc", name="acc")
            emit_comp_seq(ps, wm, wr, xm, xr,
                          slice(p * P, (p + 1) * P),
                          slice(nch * 512, (nch + 1) * 512))
            if bk_zero:
                nc.vector.tensor_copy(kp[:, nch * 512:(nch + 1) * 512], ps)
            else:
                nc.vector.tensor_scalar_add(
                    kp[:, nch * 512:(nch + 1) * 512], ps, bk_col[:, p:p + 1])
        return kp

    def emit_scores(kp, qT, p, hp, aE, psS):
        """scores + exp for head 2p+hp; returns the 8 exp tiles."""
        lo, hi = hp * DK, (hp + 1) * DK
        exps = []
        for g in range(8):
            ps = psS.tile([P, 1024], F32, tag="psS", name="psS")
            for j in range(2):
                m = 2 * g + j
                mm(ps[:, j * 512:(j + 1) * 512],
                   kp[lo:hi, m * P:(m + 1) * P],
                   qT[p][lo:hi, :], start=True, stop=True)
            e = aE.tile([P, 1024], F16, tag="exp", name="exp")
            nc.scalar.activation(e, ps, AF.Exp, scale=EXP_SCALE)
            exps.append(e)
        return exps

    def emit_p2_attn(qT, v_sb, attn3, bk_col, xk_t, aK, aE, aR, acc,
                     psS, psA, prefetch=None, pre=None):
        for p in range(H // 2):
            if p == 4 and prefetch is not None:
                nc._w1t = prefetch()
            if p == 0 and pre is not None:
                kp, pre_exps = pre
            else:
                kp, pre_exps = emit_kp(p, xk_t, aK, acc), None
            for hp in range(2):
                h = 2 * p + hp
                lo, hi = hp * DK, (hp + 1) * DK
                # one 2KB PSUM bank for all 4 q-chunk accumulators:
                # start only on the very first mm, stop only on the last.
                pa = psA.tile([P, QS, P], F32, tag="pa", name="pa")
                exps = []

                def emit_av(g):
                    e = exps[g]
                    for j in range(2):
                        m = 2 * g + j
                        for q in range(QS):
                            mm(pa[:, q, 0:DK + 1],
                               e[:, j * 512 + q * P:j * 512 + (q + 1) * P],
                               v_sb[m][:, h, :],
                               start=(g == 0 and j == 0 and q == 0),
                               stop=(g == 7 and j == 1 and q == QS - 1))

                for g in range(8):
                    ps = psS.tile([P, 1024], F32, tag="psS", name="psS")
                    for j in range(2):
                        m = 2 * g + j
                        mm(ps[:, j * 512:(j + 1) * 512],
                           kp[lo:hi, m * P:(m + 1) * P],
                           qT[p][lo:hi, :], start=True, stop=True)
                    e = aE.tile([P, 1024], F16, tag="exp", name="exp")
                    nc.scalar.activation(e, ps, AF.Exp, scale=EXP_SCALE)
                    exps.append(e)
                    if g > 0:
                        emit_av(g - 1)
                emit_av(7)
                rc = aR.tile([P, QS], F32, tag="rc", name="rc")
                nc.vector.reciprocal(rc, pa[:, :, DK:DK + 1])
                nc.vector.tensor_tensor(
                    attn3[:, :, h * DK:(h + 1) * DK], pa[:, :, 0:DK],
                    _bcast_free(rc[:, :], DK), OP.mult)

    def emit_ln_tr(attn3, ffi, ffiT8, ffiTr8, eps_t, lng_b, lnb_b, identf,
                   lnp, psTr):
        for q in range(QS):
            stats = lnp.tile([P, 2, 6], F32, tag="stats", name="stats")
            for sg in range(2):
                nc.vector.bn_stats(stats[:, sg, :],
                                   attn3[:, q, sg * 512:(sg + 1) * 512])
            mv = lnp.tile([P, 2], F32, tag="mv", name="mv")
            nc.vector.bn_aggr(mv, stats)
            std = lnp.tile([P, 1], F32, tag="std", name="std")
            nc.scalar.activation(std, mv[:, 1:2], AF.Sqrt, bias=eps_t)
            rstd = lnp.tile([P, 1], F32, tag="rstd", name="rstd")
            nc.vector.reciprocal(rstd, std)
            nc.vector.tensor_scalar(ffi[q], attn3[:, q, :], mv[:, 0:1], rstd,
                                    OP.subtract, OP.mult)
            if ln_affine:
                nc.vector.tensor_mul(ffi[q], ffi[q], lng_b)
                nc.vector.tensor_add(ffi[q], ffi[q], lnb_b)
            # transpose in f32 (walrus rejects fp8 psum outputs), then
            # fp8-split SX*ffiT out of psum on DVE
            pt = psTr.tile([P, D], F32, tag="ptr", name="ptr")
            for k in range(DCH):
                nc.tensor.transpose(pt[:, k * P:(k + 1) * P],
                                    ffi[q][:, k * P:(k + 1) * P], identf)
            ptr = pt[:].rearrange("p (a b t) -> p a b t", a=DPR, b=2)
            qsl = ffiT8[:, :, :, q * P:(q + 1) * P]
            nc.scalar.activation(qsl, ptr, AF.Copy, scale=SX)
            nc.vector.scalar_tensor_tensor(
                ffiTr8[:, :, :, q * P:(q + 1) * P], ptr, SX, qsl,
                OP.mult, OP.subtract)

    def emit_ffn(ffi, ffiT8, ffiTr8, out_sb, b1_col, b2_b,
                 w1t, w2t, hp_, h16p, psH, psF, out_dma=None):
        (w1m, w1r), (w2m, w2r) = w1t, w2t
        hT8 = hp_.tile([P, FPR, 2, T], F8, tag="hT8", name="hT8")
        hTr8 = hp_.tile([P, FPR, 2, T], F8, tag="hTr8", name="hTr8")
        pss0 = [psF.tile([P, 512], F32, tag="psF", name="psF")
                for _ in range(QS)]
        for fk in range(FF // P):
            ps = psH.tile([P, T], F32, tag="psH", name="psH")
            fsl = slice(fk * P, (fk + 1) * P)
            for k in range(DPR):
                mm(ps, w1m[k][:, :, fsl], ffiT8[:, k, :, :],
                   start=(k == 0), stop=False, perf_mode=DR)
                mm(ps, w1m[k][:, :, fsl], ffiTr8[:, k, :, :],
                   start=False, stop=False, perf_mode=DR)
                mm(ps, w1r[k][:, :, fsl], ffiT8[:, k, :, :],
                   start=False, stop=(k == DPR - 1), perf_mode=DR)
            h16 = h16p.tile([P, T], F16, tag="h16", name="h16")
            # psH = (SX*ffi)@(SW*W1) = SC*z; h16 carries SX*h = relu(psH)/SW.
            # ScalarE is idle after the exp phase -- run relu there.
            if b1_zero:
                nc.scalar.activation(h16, ps, AF.Relu, scale=1.0 / SW)
            else:
                nc.vector.tensor_scalar(h16, ps, b1_col[:, fk:fk + 1], 0.0,
                                        OP.add, OP.max)
                nc.vector.tensor_scalar_mul(h16, h16, 1.0 / SW)
            fp, sl_ = fk // 2, fk % 2
            nc.scalar.activation(hT8[:, fp, sl_, :], ps, AF.Relu,
                                 scale=1.0 / SW)
            nc.gpsimd.tensor_tensor(hTr8[:, fp, sl_, :], h16,
                                    hT8[:, fp, sl_, :], OP.subtract)
            if fk % 2 == 1:
                for q in range(QS):
                    qsl = slice(q * P, (q + 1) * P)
                    mm(pss0[q], hT8[:, fp, :, qsl], w2m[fp][:, :, 0:512],
                       start=(fp == 0), stop=False, perf_mode=DR)
                    mm(pss0[q], hTr8[:, fp, :, qsl], w2m[fp][:, :, 0:512],
                       start=False, stop=False, perf_mode=DR)
                    mm(pss0[q], hT8[:, fp, :, qsl], w2r[fp][:, :, 0:512],
                       start=False, stop=(fp == FPR - 1), perf_mode=DR)
        for q in range(QS):
            # pss carries SX*SW = SC from (SX*h)@(SW*W2)
            nc.vector.tensor_scalar_mul(out_sb[q][:, 0:512], pss0[q], 1.0 / SC)
            nc.vector.tensor_add(out_sb[q][:, 0:512], out_sb[q][:, 0:512],
                                 ffi[q][:, 0:512])
            if not b2_zero:
                nc.vector.tensor_add(out_sb[q][:, 0:512],
                                     out_sb[q][:, 0:512], b2_b[:, 0:512])
            if out_dma is not None:
                out_dma(q, 0)
        pss1 = [psF.tile([P, 512], F32, tag="psF", name="psF")
                for _ in range(QS)]
        for fp in range(FPR):
            for q in range(QS):
                qsl = slice(q * P, (q + 1) * P)
                mm(pss1[q], hT8[:, fp, :, qsl], w2m[fp][:, :, 512:1024],
                   start=(fp == 0), stop=False, perf_mode=DR)
                mm(pss1[q], hTr8[:, fp, :, qsl], w2m[fp][:, :, 512:1024],
                   start=False, stop=False, perf_mode=DR)
                mm(pss1[q], hT8[:, fp, :, qsl], w2r[fp][:, :, 512:1024],
                   start=False, stop=(fp == FPR - 1), perf_mode=DR)
        for q in range(QS):
            nc.vector.tensor_scalar_mul(out_sb[q][:, 512:1024], pss1[q],
                                        1.0 / SC)
            nc.vector.tensor_add(out_sb[q][:, 512:1024],
                                 out_sb[q][:, 512:1024],
                                 ffi[q][:, 512:1024])
            if not b2_zero:
                nc.vector.tensor_add(out_sb[q][:, 512:1024],
                                     out_sb[q][:, 512:1024],
                                     b2_b[:, 512:1024])
            if out_dma is not None:
                out_dma(q, 1)

    with tile.TileContext(nc) as tc:
        with (
            tc.tile_pool(name="const", bufs=1) as cp,
            tc.tile_pool(name="qTp", bufs=1) as qp,
            tc.tile_pool(name="attnp", bufs=1) as ap_,
            tc.tile_pool(name="fw1", bufs=1) as fw1,
            tc.tile_pool(name="fw2", bufs=1) as fw2,
            tc.tile_pool(name="accp", bufs=2, space="PSUM") as acc,
        ):
            identf = cp.tile([P, P], F32, tag="identf", name="identf")
            make_identity(nc, identf)
            eps_t = cp.tile([P, 1], F32, tag="eps", name="eps")
            nc.vector.memset(eps_t, LN_EPS)
            ones_t = cp.tile([P, H, 1], F16, tag="ones", name="ones")
            nc.vector.memset(ones_t, 1.0)
            mk = lambda shape, tag: cp.tile(shape, F32, tag=tag, name=tag)
            bq_col = None if bq_zero else mk([P, DCH], "bqc")
            bk_col = None if bk_zero else mk([P, DCH], "bkc")
            b1_col = None if b1_zero else mk([P, FF // P], "b1c")
            lng_b = mk([P, D], "lng") if ln_affine else None
            lnb_b = mk([P, D], "lnb") if ln_affine else None
            bv_b = None if bv_zero else mk([P, D], "bvb")
            b2_b = None if b2_zero else mk([P, D], "b2b")

            if not bq_zero:
                nc.sync.dma_start(bq_col, bq[:].rearrange("(o p) -> p o", p=P))
            if not bk_zero:
                nc.sync.dma_start(bk_col, bk[:].rearrange("(o p) -> p o", p=P))
            if not b1_zero:
                nc.sync.dma_start(b1_col, b1[:].rearrange("(o p) -> p o", p=P))
            if ln_affine:
                nc.gpsimd.dma_start(lng_b, _bcast_ap(ln_g[:]))
                nc.gpsimd.dma_start(lnb_b, _bcast_ap(ln_b[:]))
            if not bv_zero:
                nc.gpsimd.dma_start(bv_b, _bcast_ap(bv[:]))
            if not b2_zero:
                nc.gpsimd.dma_start(b2_b, _bcast_ap(b2[:]))

            qT = [qp.tile([P, T], F16, tag=f"qT{m}", name=f"qT{m}")
                  for m in range(DCH)]
            attn3 = ap_.tile([P, QS, D], F32, tag="attn3", name="attn3")

            with tc.tile_pool(name="p1w", bufs=1) as p1w:
                emit_p1(qT, bq_col, p1w, acc)

            with tc.tile_pool(name="vp", bufs=1) as vp:
                v_sb = [vp.tile([P, H, DK + 1], F16, tag=f"v{t}",
                                name=f"v{t}")
                        for t in range(KC)]
                with tc.tile_pool(name="p2w", bufs=1) as p2w:
                    with (
                        tc.tile_pool(name="p3w", bufs=1) as p3w,
                        tc.tile_pool(name="p3x", bufs=2) as p3x,
                    ):
                        emit_p3(v_sb, bv_b, ones_t, p3w, p3x, acc,
                                prefetch=lambda h: load_xk_wk(p2w, h))
                        xk_t = nc._xkwk

                    with (
                        tc.tile_pool(name="aK", bufs=3) as aK,
                        tc.tile_pool(name="aE", bufs=12) as aE,
                        tc.tile_pool(name="aR", bufs=2) as aR,
                        tc.tile_pool(name="psS", bufs=2, space="PSUM") as psS,
                        tc.tile_pool(name="psA", bufs=2, space="PSUM") as psA,
                    ):
                        emit_p2_attn(qT, v_sb, attn3, bk_col, xk_t,
                                     aK, aE, aR, acc, psS, psA,
                                     prefetch=lambda: load_pairs(
                                         fw1, "w1", w18, w1r8, FF))
                        w1t = nc._w1t
                        # prewarm the Sqrt ACT table set so the switch
                        # isn't on the LayerNorm critical path
                        warm = aR.tile([P, 1], F32, tag="warm", name="warm")
                        nc.scalar.activation(warm, eps_t, AF.Sqrt)

            with (
                tc.tile_pool(name="ffip", bufs=1) as fip,
                tc.tile_pool(name="ffiTp", bufs=1) as ftp,
                tc.tile_pool(name="outp", bufs=1) as op_,
            ):
                ffi = [fip.tile([P, D], F32, tag=f"ffi{q}", name=f"ffi{q}")
                       for q in range(QS)]
                ffiT8 = ftp.tile([P, DPR, 2, T], F8, tag="ffiT8",
                                 name="ffiT8")
                ffiTr8 = ftp.tile([P, DPR, 2, T], F8, tag="ffiTr8",
                                  name="ffiTr8")
                out_sb = [op_.tile([P, D], F32, tag=f"out{q}", name=f"out{q}")
                          for q in range(QS)]

                w2t = load_pairs(fw2, "w2", w28, w2r8, D, npairs=FPR)
                with (
                    tc.tile_pool(name="lnp", bufs=4) as lnp,
                    tc.tile_pool(name="psTr", bufs=3, space="PSUM") as psTr,
                ):
                    emit_ln_tr(attn3, ffi, ffiT8, ffiTr8, eps_t, lng_b,
                               lnb_b, identf, lnp, psTr)

                with (
                    tc.tile_pool(name="hTp", bufs=1) as hp_,
                    tc.tile_pool(name="h16p", bufs=4) as h16p,
                    tc.tile_pool(name="psH", bufs=2, space="PSUM") as psH,
                    tc.tile_pool(name="psF", bufs=4, space="PSUM") as psF,
                ):
                    def out_dma(q, half):
                        sl = slice(half * 512, (half + 1) * 512)
                        nc.sync.dma_start(out[q * P:(q + 1) * P, sl],
                                          out_sb[q][:, sl])
                    emit_ffn(ffi, ffiT8, ffiTr8, out_sb, b1_col, b2_b,
                             w1t, w2t, hp_, h16p, psH, psF, out_dma=out_dma)

    nc.compile()
    return nc


def classify_inputs(inputs):
    f32 = lambda a: np.asarray(a, dtype=np.float32)
    return dict(
        ln_affine=not (np.all(f32(inputs["ln_g"]) == 1.0)
                       and np.all(f32(inputs["ln_b"]) == 0.0)),
        bq_zero=not f32(inputs["bq"]).any(),
        bk_zero=not f32(inputs["bk"]).any(),
        bv_zero=not f32(inputs["bv"]).any(),
        b1_zero=not f32(inputs["b1"]).any(),
        b2_zero=not f32(inputs["b2"]).any(),
    )


def _split8(a, scale):
    """Host-side compensated fp8 split: returns (f8(scale*a), f8(residual))."""
    import ml_dtypes
    E4 = ml_dtypes.float8_e4m3
    s = np.asarray(a, np.float32) * scale
    m = s.astype(E4)
    r = (s - m.astype(np.float32)).astype(E4)
    return np.ascontiguousarray(m), np.ascontiguousarray(r)


def build_in_maps(inputs):
    f32 = lambda a: np.asarray(a, dtype=np.float32)
    query, key, value = f32(inputs["query"]), f32(inputs["key"]), f32(inputs["value"])

    wq8, wqr8 = _split8(inputs["Wq"], SW)
    wk8, wkr8 = _split8(inputs["Wk"], SW)
    wv8, wvr8 = _split8(inputs["Wv"], SW)
    w18, w1r8 = _split8(inputs["W1"], SW)
    w28, w2r8 = _split8(inputs["W2"], SW)
    shared = dict(
        wq8=wq8, wq8r=wqr8, wk8=wk8, wk8r=wkr8, wv8=wv8, wv8r=wvr8,
        w18=w18, w18r=w1r8, w28=w28, w28r=w2r8,
        bq=f32(inputs["bq"]) * SC, bk=f32(inputs["bk"]) * SC,
        bv=f32(inputs["bv"]) * SC, b1=f32(inputs["b1"]) * SC,
        b2=f32(inputs["b2"]),
        ln_g=f32(inputs["ln_g"]), ln_b=f32(inputs["ln_b"]),
    )

    in_maps = []
    for c in range(N_CORES):
        b = c // 4
        t0 = (c % 4) * T
        xq8, xqr8 = _split8(query[b, t0:t0 + T, :].T, SX)
        xk8, xkr8 = _split8(key[b].T, SX)
        xv8, xvr8 = _split8(value[b].T, SX)
        in_maps.append(dict(
            xq8=xq8, xq8r=xqr8, xk8=xk8, xk8r=xkr8, xv8=xv8, xv8r=xvr8,
            **shared,
        ))
    return in_maps


def kernel(**inputs) -> np.ndarray:
    nc = build_program(**classify_inputs(inputs))
    in_maps = build_in_maps(inputs)
    res = run_bass_kernel_spmd(nc, in_maps, list(range(N_CORES)))
    out = np.empty((B, S, D), dtype=np.float32)
    for c in range(N_CORES):
        b = c // 4
        t0 = (c % 4) * T
        out[b, t0:t0 + T, :] = res.results[c]["out"]
    return out


# revision 54
# speedup vs baseline: 1.0106x; 1.0106x over previous
"""Trainium2 Bass kernel for nn_MultiHeadAttention_36223754174786.

Fused transformer block: QKV projection -> 16-head attention (naive, full
[S,S] scores) -> LayerNorm -> FeedForward (relu MLP) with residual.
B=2, S=2048, D=1024, H=16, DK=64, FF_HIDDEN=2048.

Sharding: data-parallel over tokens across 8 NeuronCores.  Core c handles 512
query tokens of batch b=c//4.  K/V projections for the full batch are
recomputed on each core: cross-core exchange is infeasible here (remote_dma
deadlocks the Tile scheduler's sim; collective_compute is priced 15us+40GB/s
in the cost model), so no cross-core communication at all.

Numerics -- compensated fp8 DoubleRow (the workhorse):
  A plain fp8e4 operand costs ~1.2e-2 on the rel-err metric (the metric is
  brutally sensitive: attention output pre-LN has std ~0.02, so LayerNorm
  amplifies any attn-path error ~50x).  Instead, every host-prepared matmul
  (Q/K/V projections, FFN W1/W2) runs THREE fp8-DR products per chunk-pair:
    x@w ~= x8@w8 + xr8@w8 + x8@wr8,   x8=f8(SX*x), xr8=f8(SX*x - x8),
                                      w8=f8(SW*w), wr8=f8(SW*w - w8)
  with SX=4, SW=256 chosen so mains AND residuals sit in fp8's normal range.
  Residual-compensation leaves only the xr*wr cross term (~0.1%, measured
  1.3e-3 end-to-end).  Cost: 3 DR mms (0.5 c/r each) per pair = 0.75x the
  fp16 cost; fp16 elsewhere (scores, attnV, exp) -- fp16 is priced like bf16
  (1.0 c/r) but has 8x finer mantissa, so it's free accuracy.
  Every activation in the fused chain then carries a uniform SC=SX*SW=1024
  scale: exp() folds it away via its scale operand (1/(SC^2*sqrt(dk))),
  LayerNorm is scale-invariant BUT its epsilon must be scaled by SC^2
  (1e-5 at native scale is a 2.4% variance perturbation -- dropping it
  costs 2e-2 on the metric!), and the FFN residual-add unscales by 1/SC.

Attention layout (vs the 353us baseline):
  scores are produced TRANSPOSED, sT[keys,q] (16 psum chunks per head),
  exp'd straight out of PSUM on ScalarE into fp16 SBUF tiles e[keys,2*512q].
  attnV computes token-major out[q, dk] per head with lhsT=e (128-col
  slices) and rhs=[V_h | 1] token-major [keys, 65]: moving dim 65 instead
  of 512 -- and row 64 accumulates the softmax denominator, the output
  lands token-major, so NO PE transposes or per-q reciprocal loops.  The 4
  q-chunk accumulators share one PSUM bank (start once, stop once).

PE rows/core: Qp 24.6k + Kp 98.3k + Vp 98.3k + scores 131k + attnV 66.5k +
W1 49.1k + W2 49.1k + LN-tr 8.2k ~= 525k ~= 226us busy; ScalarE exp ~137us.
Timeline-sim total: 278.3us (was 353.5us baseline); rel err 1.45e-3.
The xk/wk prefetch is split across two p3 token-groups: the attention phase
is ScalarE-bound (137us of exp), so its start is gated by these DMA arrivals
competing with the xv stream for bandwidth.
Scores cannot go compensated-fp8: contraction is only 64 deep, so DoubleRow
gives no depth advantage and 3 products cost 1.5x the single fp16 matmul.
"""

import numpy as np

import concourse.bass as bass
import concourse.tile as tile
from concourse import bacc, mybir
from concourse.bass_utils import run_bass_kernel_spmd
from concourse.masks import make_identity

F32 = mybir.dt.float32
F16 = mybir.dt.float16
F8 = mybir.dt.float8e4
DR = mybir.MatmulPerfMode.DoubleRow
AF = mybir.ActivationFunctionType
OP = mybir.AluOpType

B, S, D, H = 2, 2048, 1024, 16
DK = D // H          # 64
FF = 2048
P = 128
T = 512              # query tokens per core
N_CORES = 8
KC = S // P          # 16 key chunks
QS = T // P          # 4 query sub-tiles
DCH = D // P         # 8 chunks of the model dim
DPR = DCH // 2       # 4 fp8 pair-chunks of the model dim
FPR = FF // 256      # 8 fp8 pair-chunks of the ffn hidden dim
SX = 4.0             # fp8 scale, activation side
SW = 256.0           # fp8 scale, weight side
SC = SX * SW         # scale carried by every projected activation
EXP_SCALE = 1.0 / (SC * SC * 8.0)   # un-scales q*k and folds 1/sqrt(dk)
LN_EPS = 1e-5 * SC * SC             # attn3 carries SC => var carries SC^2


def _bcast_ap(ap):
    """Partition-broadcast a 1-D DRAM vector to [128, n] for DMA."""
    return bass.AP(tensor=ap.tensor, offset=ap.offset, ap=[[0, P]] + list(ap.ap))


def _bcast_free(ap, n):
    """Append a stride-0 free dim of size n (broadcast-read for DVE ops)."""
    return bass.AP(tensor=ap.tensor, offset=ap.offset,
                   ap=list(ap.ap) + [[0, n]])


def build_program(ln_affine=True, bq_zero=False, bk_zero=False, bv_zero=False,
                  b1_zero=False, b2_zero=False):
    nc = bacc.Bacc("TRN2", target_bir_lowering=False, debug=False,
                   num_devices=N_CORES)

    def mm(out_ap, lhsT, rhs, start, stop, perf_mode=None):
        nc.tensor.matmul(out_ap, lhsT, rhs, start=start, stop=stop,
                         perf_mode=perf_mode)

    def dram_pair(name, shape):
        """main + residual fp8 DRAM inputs."""
        return (nc.dram_tensor(name, shape, F8, kind="ExternalInput"),
                nc.dram_tensor(name + "r", shape, F8, kind="ExternalInput"))

    xq8, xqr8 = dram_pair("xq8", [D, T])
    wq8, wqr8 = dram_pair("wq8", [D, D])
    xk8, xkr8 = dram_pair("xk8", [D, S])
    wk8, wkr8 = dram_pair("wk8", [D, D])
    xv8, xvr8 = dram_pair("xv8", [D, S])
    wv8, wvr8 = dram_pair("wv8", [D, D])
    w18, w1r8 = dram_pair("w18", [D, FF])
    w28, w2r8 = dram_pair("w28", [FF, D])
    bq = nc.dram_tensor("bq", [D], F32, kind="ExternalInput")
    bk = nc.dram_tensor("bk", [D], F32, kind="ExternalInput")
    bv = nc.dram_tensor("bv", [D], F32, kind="ExternalInput")
    b1 = nc.dram_tensor("b1", [FF], F32, kind="ExternalInput")
    b2 = nc.dram_tensor("b2", [D], F32, kind="ExternalInput")
    ln_g = nc.dram_tensor("ln_g", [D], F32, kind="ExternalInput")
    ln_b = nc.dram_tensor("ln_b", [D], F32, kind="ExternalInput")
    out = nc.dram_tensor("out", [T, D], F32, kind="ExternalOutput")

    def load_pairs(pool, tag, main, resid, cols, col0=0, npairs=DPR,
                   eng=None, pair0=0):
        """fp8 pair-tiles [P, 2, cols]: slot i of pair k holds DRAM rows
        256k+128i.., so [:, :, slice] is the DR [Ki, 2, dim] layout.
        Returns (mains, resids), each a list of npairs tiles."""
        eng = eng or nc.sync
        ms, rs = [], []
        for k in range(pair0, pair0 + npairs):
            for lst, src, nm in ((ms, main, tag), (rs, resid, tag + "r")):
                t_ = pool.tile([P, 2, cols], F8, tag=f"{nm}{k}",
                               name=f"{nm}{k}")
                eng.dma_start(
                    t_,
                    src[256 * k:256 * (k + 1), col0:col0 + cols].rearrange(
                        "(i p) c -> p i c", i=2))
                lst.append(t_)
        return ms, rs

    def emit_comp(ps, wm, wr, xm, xr, wsl, xsl, start):
        """One chunk-pair of the compensated product into psum `ps`:
        w8.T@x8 + w8.T@xr8 + wr8.T@x8, each a DoubleRow matmul."""
        mm(ps, wm[:, :, wsl], xm[:, :, xsl], start=start, stop=False,
           perf_mode=DR)
        mm(ps, wm[:, :, wsl], xr[:, :, xsl], start=False, stop=False,
           perf_mode=DR)
        return lambda stop: mm(ps, wr[:, :, wsl], xm[:, :, xsl], start=False,
                               stop=stop, perf_mode=DR)

    def emit_comp_seq(ps, wms, wrs, xms, xrs, wsl, xsl):
        """Full contraction (DPR pairs x 3 products) into psum `ps`."""
        for k in range(DPR):
            last = emit_comp(ps, wms[k], wrs[k], xms[k], xrs[k], wsl, xsl,
                             start=(k == 0))
            last(stop=(k == DPR - 1))

    def emit_p1(qT, bq_col, p1w, acc):
        xm, xr = load_pairs(p1w, "xq", xq8, xqr8, T)
        wm, wr = load_pairs(p1w, "wq", wq8, wqr8, D)
        for m in range(DCH):
            ps = acc.tile([P, 512], F32, tag="acc", name="acc")
            emit_comp_seq(ps, wm, wr, xm, xr,
                          slice(m * P, (m + 1) * P), slice(None))
            if bq_zero:
                nc.vector.tensor_copy(qT[m], ps)
            else:
                nc.vector.tensor_scalar_add(qT[m], ps, bq_col[:, m:m + 1])

    def emit_p3(v_sb, bv_b, ones_t, p3w, p3x, acc, prefetch=None,
                prefetch2=None):
        wm, wr = load_pairs(p3w, "wv", wv8, wvr8, D)
        for tg in range(KC // 4):
            xm, xr = load_pairs(p3x, "xv", xv8, xvr8, 512, col0=tg * 512)
            if tg in (1, 2) and prefetch is not None:
                nc._xkwk = prefetch(tg - 1)
            if tg == 2 and prefetch2 is not None:
                prefetch2()
            for t in range(tg * 4, tg * 4 + 4):
                nc.vector.tensor_copy(v_sb[t][:, :, DK:DK + 1], ones_t)
                for dch in range(2):
                    ps = acc.tile([P, 512], F32, tag="acc", name="acc")
                    # token-major: lhsT = x pairs, rhs = w pairs
                    for k in range(DPR):
                        tsl = slice((t - tg * 4) * P, (t - tg * 4 + 1) * P)
                        csl = slice(dch * 512, (dch + 1) * 512)
                        mm(ps, xm[k][:, :, tsl], wm[k][:, :, csl],
                           start=(k == 0), stop=False, perf_mode=DR)
                        mm(ps, xr[k][:, :, tsl], wm[k][:, :, csl],
                           start=False, stop=False, perf_mode=DR)
                        mm(ps, xm[k][:, :, tsl], wr[k][:, :, csl],
                           start=False, stop=(k == DPR - 1), perf_mode=DR)
                    if bv_zero:
                        nc.vector.tensor_copy(
                            v_sb[t][:, dch * 8:(dch + 1) * 8, 0:DK],
                            ps[:].rearrange("p (h d) -> p h d", h=8))
                    else:
                        nc.vector.tensor_tensor(
                            v_sb[t][:, dch * 8:(dch + 1) * 8, 0:DK],
                            ps[:].rearrange("p (h d) -> p h d", h=8),
                            bv_b[:, dch * 512:(dch + 1) * 512].rearrange(
                                "p (h d) -> p h d", h=8),
                            OP.add)

    def load_xk_wk(p2w, part):
        """two parts (p3 tg 1 and 2) to smooth DMA contention with xv"""
        if part == 0:
            nc._xkh0 = load_pairs(p2w, "xk", xk8, xkr8, S, npairs=2)
            return None
        xm2, xr2 = load_pairs(p2w, "xkB", xk8, xkr8, S, npairs=2, pair0=2)
        wm, wr = load_pairs(p2w, "wk", wk8, wkr8, D)
        xm, xr = nc._xkh0
        return xm + xm2, xr + xr2, wm, wr

    def emit_kp(p, xk_t, aK, acc):
        xm, xr, wm, wr = xk_t
        kp = aK.tile([P, S], F16, tag="kp", name="kp")
        for nch in range(S // 512):
            ps = acc.tile([P, 512], F32, tag="acc", name="acc")
            emit_comp_seq(ps, wm, wr, xm, xr,
                          slice(p * P, (p + 1) * P),
                          slice(nch * 512, (nch + 1) * 512))
            if bk_zero:
                nc.vector.tensor_copy(kp[:, nch * 512:(nch + 1) * 512], ps)
            else:
                nc.vector.tensor_scalar_add(
                    kp[:, nch * 512:(nch + 1) * 512], ps, bk_col[:, p:p + 1])
        return kp

    def emit_scores(kp, qT, p, hp, aE, psS):
        """scores + exp for head 2p+hp; returns the 8 exp tiles."""
        lo, hi = hp * DK, (hp + 1) * DK
        exps = []
        for g in range(8):
            ps = psS.tile([P, 1024], F32, tag="psS", name="psS")
            for j in range(2):
                m = 2 * g + j
                mm(ps[:, j * 512:(j + 1) * 512],
                   kp[lo:hi, m * P:(m + 1) * P],
                   qT[p][lo:hi, :], start=True, stop=True)
            e = aE.tile([P, 1024], F16, tag="exp", name="exp")
            nc.scalar.activation(e, ps, AF.Exp, scale=EXP_SCALE)
            exps.append(e)
        return exps

    def emit_p2_attn(qT, v_sb, attn3, bk_col, xk_t, aK, aE, aR, acc,
                     psS, psA, prefetch=None, pre=None):
        for p in range(H // 2):
            if p == 4 and prefetch is not None:
                nc._w1t = prefetch()
            if p == 0 and pre is not None:
                kp, pre_exps = pre
            else:
                kp, pre_exps = emit_kp(p, xk_t, aK, acc), None
            for hp in range(2):
                h = 2 * p + hp
                lo, hi = hp * DK, (hp + 1) * DK
                # one 2KB PSUM bank for all 4 q-chunk accumulators:
                # start only on the very first mm, stop only on the last.
                pa = psA.tile([P, QS, P], F32, tag="pa", name="pa")
                exps = []

                def emit_av(g):
                    e = exps[g]
                    for j in range(2):
                        m = 2 * g + j
                        for q in range(QS):
                            mm(pa[:, q, 0:DK + 1],
                               e[:, j * 512 + q * P:j * 512 + (q + 1) * P],
                               v_sb[m][:, h, :],
                               start=(g == 0 and j == 0 and q == 0),
                               stop=(g == 7 and j == 1 and q == QS - 1))

                for g in range(8):
                    ps = psS.tile([P, 1024], F32, tag="psS", name="psS")
                    for j in range(2):
                        m = 2 * g + j
                        mm(ps[:, j * 512:(j + 1) * 512],
                           kp[lo:hi, m * P:(m + 1) * P],
                           qT[p][lo:hi, :], start=True, stop=True)
                    e = aE.tile([P, 1024], F16, tag="exp", name="exp")
                    nc.scalar.activation(e, ps, AF.Exp, scale=EXP_SCALE)
                    exps.append(e)
                    if g > 0:
                        emit_av(g - 1)
                emit_av(7)
                rc = aR.tile([P, QS], F32, tag="rc", name="rc")
                nc.vector.reciprocal(rc, pa[:, :, DK:DK + 1])
                nc.vector.tensor_tensor(
                    attn3[:, :, h * DK:(h + 1) * DK], pa[:, :, 0:DK],
                    _bcast_free(rc[:, :], DK), OP.mult)

    def emit_ln_tr(attn3, ffi, ffiT8, ffiTr8, eps_t, lng_b, lnb_b, identf,
                   lnp, psTr):
        for q in range(QS):
            stats = lnp.tile([P, 2, 6], F32, tag="stats", name="stats")
            for sg in range(2):
                nc.vector.bn_stats(stats[:, sg, :],
                                   attn3[:, q, sg * 512:(sg + 1) * 512])
            mv = lnp.tile([P, 2], F32, tag="mv", name="mv")
            nc.vector.bn_aggr(mv, stats)
            std = lnp.tile([P, 1], F32, tag="std", name="std")
            nc.scalar.activation(std, mv[:, 1:2], AF.Sqrt, bias=eps_t)
            rstd = lnp.tile([P, 1], F32, tag="rstd", name="rstd")
            nc.vector.reciprocal(rstd, std)
            nc.gpsimd.tensor_scalar(ffi[q], attn3[:, q, :], mv[:, 0:1], rstd,
                                     OP.subtract, OP.mult)
            if ln_affine:
                nc.vector.tensor_mul(ffi[q], ffi[q], lng_b)
                nc.vector.tensor_add(ffi[q], ffi[q], lnb_b)
            # transpose in f32 (walrus rejects fp8 psum outputs), then
            # fp8-split SX*ffiT out of psum on DVE
            pt = psTr.tile([P, D], F32, tag="ptr", name="ptr")
            for k in range(DCH):
                nc.tensor.transpose(pt[:, k * P:(k + 1) * P],
                                    ffi[q][:, k * P:(k + 1) * P], identf)
            ptr = pt[:].rearrange("p (a b t) -> p a b t", a=DPR, b=2)
            qsl = ffiT8[:, :, :, q * P:(q + 1) * P]
            nc.scalar.activation(qsl, ptr, AF.Copy, scale=SX)
            nc.vector.scalar_tensor_tensor(
                ffiTr8[:, :, :, q * P:(q + 1) * P], ptr, SX, qsl,
                OP.mult, OP.subtract)

    def emit_ffn(ffi, ffiT8, ffiTr8, out_sb, b1_col, b2_b,
                 w1t, w2t, hp_, h16p, psH, psF, out_dma=None):
        (w1m, w1r), (w2m, w2r) = w1t, w2t
        hT8 = hp_.tile([P, FPR, 2, T], F8, tag="hT8", name="hT8")
        hTr8 = hp_.tile([P, FPR, 2, T], F8, tag="hTr8", name="hTr8")
        pss0 = [psF.tile([P, 512], F32, tag="psF", name="psF")
                for _ in range(QS)]
        for fk in range(FF // P):
            ps = psH.tile([P, T], F32, tag="psH", name="psH")
            fsl = slice(fk * P, (fk + 1) * P)
            for k in range(DPR):
                mm(ps, w1m[k][:, :, fsl], ffiT8[:, k, :, :],
                   start=(k == 0), stop=False, perf_mode=DR)
                mm(ps, w1m[k][:, :, fsl], ffiTr8[:, k, :, :],
                   start=False, stop=False, perf_mode=DR)
                mm(ps, w1r[k][:, :, fsl], ffiT8[:, k, :, :],
                   start=False, stop=(k == DPR - 1), perf_mode=DR)
            h16 = h16p.tile([P, T], F16, tag="h16", name="h16")
            # psH = (SX*ffi)@(SW*W1) = SC*z; h16 carries SX*h = relu(psH)/SW.
            # ScalarE is idle after the exp phase -- run relu there.
            if b1_zero:
                nc.scalar.activation(h16, ps, AF.Relu, scale=1.0 / SW)
            else:
                nc.vector.tensor_scalar(h16, ps, b1_col[:, fk:fk + 1], 0.0,
                                        OP.add, OP.max)
                nc.vector.tensor_scalar_mul(h16, h16, 1.0 / SW)
            fp, sl_ = fk // 2, fk % 2
            nc.scalar.activation(hT8[:, fp, sl_, :], ps, AF.Relu,
                                 scale=1.0 / SW)
            nc.gpsimd.tensor_tensor(hTr8[:, fp, sl_, :], h16,
                                    hT8[:, fp, sl_, :], OP.subtract)
            if fk % 2 == 1:
                for q in range(QS):
                    qsl = slice(q * P, (q + 1) * P)
                    mm(pss0[q], hT8[:, fp, :, qsl], w2m[fp][:, :, 0:512],
                       start=(fp == 0), stop=False, perf_mode=DR)
                    mm(pss0[q], hTr8[:, fp, :, qsl], w2m[fp][:, :, 0:512],
                       start=False, stop=False, perf_mode=DR)
                    mm(pss0[q], hT8[:, fp, :, qsl], w2r[fp][:, :, 0:512],
                       start=False, stop=(fp == FPR - 1), perf_mode=DR)
        for q in range(QS):
            # pss carries SX*SW = SC from (SX*h)@(SW*W2)
            nc.vector.tensor_scalar_mul(out_sb[q][:, 0:512], pss0[q], 1.0 / SC)
            nc.vector.tensor_add(out_sb[q][:, 0:512], out_sb[q][:, 0:512],
                                 ffi[q][:, 0:512])
            if not b2_zero:
                nc.vector.tensor_add(out_sb[q][:, 0:512],
                                     out_sb[q][:, 0:512], b2_b[:, 0:512])
            if out_dma is not None:
                out_dma(q, 0)
        pss1 = [psF.tile([P, 512], F32, tag="psF", name="psF")
                for _ in range(QS)]
        for fp in range(FPR):
            for q in range(QS):
                qsl = slice(q * P, (q + 1) * P)
                mm(pss1[q], hT8[:, fp, :, qsl], w2m[fp][:, :, 512:1024],
                   start=(fp == 0), stop=False, perf_mode=DR)
                mm(pss1[q], hTr8[:, fp, :, qsl], w2m[fp][:, :, 512:1024],
                   start=False, stop=False, perf_mode=DR)
                mm(pss1[q], hT8[:, fp, :, qsl], w2r[fp][:, :, 512:1024],
                   start=False, stop=(fp == FPR - 1), perf_mode=DR)
        for q in range(QS):
            nc.vector.tensor_scalar_mul(out_sb[q][:, 512:1024], pss1[q],
                                        1.0 / SC)
            nc.vector.tensor_add(out_sb[q][:, 512:1024],
                                 out_sb[q][:, 512:1024],
                                 ffi[q][:, 512:1024])
            if not b2_zero:
                nc.vector.tensor_add(out_sb[q][:, 512:1024],
                                     out_sb[q][:, 512:1024],
                                     b2_b[:, 512:1024])
            if out_dma is not None:
                out_dma(q, 1)

    with tile.TileContext(nc) as tc:
        with (
            tc.tile_pool(name="const", bufs=1) as cp,
            tc.tile_pool(name="qTp", bufs=1) as qp,
            tc.tile_pool(name="attnp", bufs=1) as ap_,
            tc.tile_pool(name="fw1", bufs=1) as fw1,
            tc.tile_pool(name="fw2", bufs=1) as fw2,
            tc.tile_pool(name="accp", bufs=2, space="PSUM") as acc,
        ):
            identf = cp.tile([P, P], F32, tag="identf", name="identf")
            make_identity(nc, identf)
            eps_t = cp.tile([P, 1], F32, tag="eps", name="eps")
            nc.vector.memset(eps_t, LN_EPS)
            ones_t = cp.tile([P, H, 1], F16, tag="ones", name="ones")
            nc.vector.memset(ones_t, 1.0)
            mk = lambda shape, tag: cp.tile(shape, F32, tag=tag, name=tag)
            bq_col = None if bq_zero else mk([P, DCH], "bqc")
            bk_col = None if bk_zero else mk([P, DCH], "bkc")
            b1_col = None if b1_zero else mk([P, FF // P], "b1c")
            lng_b = mk([P, D], "lng") if ln_affine else None
            lnb_b = mk([P, D], "lnb") if ln_affine else None
            bv_b = None if bv_zero else mk([P, D], "bvb")
            b2_b = None if b2_zero else mk([P, D], "b2b")

            if not bq_zero:
                nc.sync.dma_start(bq_col, bq[:].rearrange("(o p) -> p o", p=P))
            if not bk_zero:
                nc.sync.dma_start(bk_col, bk[:].rearrange("(o p) -> p o", p=P))
            if not b1_zero:
                nc.sync.dma_start(b1_col, b1[:].rearrange("(o p) -> p o", p=P))
            if ln_affine:
                nc.gpsimd.dma_start(lng_b, _bcast_ap(ln_g[:]))
                nc.gpsimd.dma_start(lnb_b, _bcast_ap(ln_b[:]))
            if not bv_zero:
                nc.gpsimd.dma_start(bv_b, _bcast_ap(bv[:]))
            if not b2_zero:
                nc.gpsimd.dma_start(b2_b, _bcast_ap(b2[:]))

            qT = [qp.tile([P, T], F16, tag=f"qT{m}", name=f"qT{m}")
                  for m in range(DCH)]
            attn3 = ap_.tile([P, QS, D], F32, tag="attn3", name="attn3")

            with tc.tile_pool(name="p1w", bufs=1) as p1w:
                emit_p1(qT, bq_col, p1w, acc)

            with tc.tile_pool(name="vp", bufs=1) as vp:
                v_sb = [vp.tile([P, H, DK + 1], F16, tag=f"v{t}",
                                name=f"v{t}")
                        for t in range(KC)]
                with tc.tile_pool(name="p2w", bufs=1) as p2w:
                    with (
                        tc.tile_pool(name="p3w", bufs=1) as p3w,
                        tc.tile_pool(name="p3x", bufs=2) as p3x,
                    ):
                        emit_p3(v_sb, bv_b, ones_t, p3w, p3x, acc,
                                prefetch=lambda h: load_xk_wk(p2w, h))
                        xk_t = nc._xkwk

                    with (
                        tc.tile_pool(name="aK", bufs=3) as aK,
                        tc.tile_pool(name="aE", bufs=12) as aE,
                        tc.tile_pool(name="aR", bufs=2) as aR,
                        tc.tile_pool(name="psS", bufs=2, space="PSUM") as psS,
                        tc.tile_pool(name="psA", bufs=2, space="PSUM") as psA,
                    ):
                        emit_p2_attn(qT, v_sb, attn3, bk_col, xk_t,
                                     aK, aE, aR, acc, psS, psA,
                                     prefetch=lambda: load_pairs(
                                         fw1, "w1", w18, w1r8, FF))
                        w1t = nc._w1t
                        # prewarm the Sqrt ACT table set so the switch
                        # isn't on the LayerNorm critical path
                        warm = aR.tile([P, 1], F32, tag="warm", name="warm")
                        nc.scalar.activation(warm, eps_t, AF.Sqrt)

            with (
                tc.tile_pool(name="ffip", bufs=1) as fip,
                tc.tile_pool(name="ffiTp", bufs=1) as ftp,
                tc.tile_pool(name="outp", bufs=1) as op_,
            ):
                ffi = [fip.tile([P, D], F32, tag=f"ffi{q}", name=f"ffi{q}")
                       for q in range(QS)]
                ffiT8 = ftp.tile([P, DPR, 2, T], F8, tag="ffiT8",
                                 name="ffiT8")
                ffiTr8 = ftp.tile([P, DPR, 2, T], F8, tag="ffiTr8",
                                  name="ffiTr8")
                out_sb = [op_.tile([P, D], F32, tag=f"out{q}", name=f"out{q}")
                          for q in range(QS)]

                w2t = load_pairs(fw2, "w2", w28, w2r8, D, npairs=FPR)
                with (
                    tc.tile_pool(name="lnp", bufs=4) as lnp,
                    tc.tile_pool(name="psTr", bufs=4, space="PSUM") as psTr,
                ):
                    emit_ln_tr(attn3, ffi, ffiT8, ffiTr8, eps_t, lng_b,
                               lnb_b, identf, lnp, psTr)

                with (
                    tc.tile_pool(name="hTp", bufs=1) as hp_,
                    tc.tile_pool(name="h16p", bufs=4) as h16p,
                    tc.tile_pool(name="psH", bufs=4, space="PSUM") as psH,
                    tc.tile_pool(name="psF", bufs=4, space="PSUM") as psF,
                ):
                    def out_dma(q, half):
                        sl = slice(half * 512, (half + 1) * 512)
                        nc.sync.dma_start(out[q * P:(q + 1) * P, sl],
                                          out_sb[q][:, sl])
                    emit_ffn(ffi, ffiT8, ffiTr8, out_sb, b1_col, b2_b,
                             w1t, w2t, hp_, h16p, psH, psF, out_dma=out_dma)

    nc.compile()
    return nc


def classify_inputs(inputs):
    f32 = lambda a: np.asarray(a, dtype=np.float32)
    return dict(
        ln_affine=not (np.all(f32(inputs["ln_g"]) == 1.0)
                       and np.all(f32(inputs["ln_b"]) == 0.0)),
        bq_zero=not f32(inputs["bq"]).any(),
        bk_zero=not f32(inputs["bk"]).any(),
        bv_zero=not f32(inputs["bv"]).any(),
        b1_zero=not f32(inputs["b1"]).any(),
        b2_zero=not f32(inputs["b2"]).any(),
    )


def _split8(a, scale):
    """Host-side compensated fp8 split: returns (f8(scale*a), f8(residual))."""
    import ml_dtypes
    E4 = ml_dtypes.float8_e4m3
    s = np.asarray(a, np.float32) * scale
    m = s.astype(E4)
    r = (s - m.astype(np.float32)).astype(E4)
    return np.ascontiguousarray(m), np.ascontiguousarray(r)


def build_in_maps(inputs):
    f32 = lambda a: np.asarray(a, dtype=np.float32)
    query, key, value = f32(inputs["query"]), f32(inputs["key"]), f32(inputs["value"])

    wq8, wqr8 = _split8(inputs["Wq"], SW)
    wk8, wkr8 = _split8(inputs["Wk"], SW)
    wv8, wvr8 = _split8(inputs["Wv"], SW)
    w18, w1r8 = _split8(inputs["W1"], SW)
    w28, w2r8 = _split8(inputs["W2"], SW)
    shared = dict(
        wq8=wq8, wq8r=wqr8, wk8=wk8, wk8r=wkr8, wv8=wv8, wv8r=wvr8,
        w18=w18, w18r=w1r8, w28=w28, w28r=w2r8,
        bq=f32(inputs["bq"]) * SC, bk=f32(inputs["bk"]) * SC,
        bv=f32(inputs["bv"]) * SC, b1=f32(inputs["b1"]) * SC,
        b2=f32(inputs["b2"]),
        ln_g=f32(inputs["ln_g"]), ln_b=f32(inputs["ln_b"]),
    )

    in_maps = []
    for c in range(N_CORES):
        b = c // 4
        t0 = (c % 4) * T
        xq8, xqr8 = _split8(query[b, t0:t0 + T, :].T, SX)
        xk8, xkr8 = _split8(key[b].T, SX)
        xv8, xvr8 = _split8(value[b].T, SX)
        in_maps.append(dict(
            xq8=xq8, xq8r=xqr8, xk8=xk8, xk8r=xkr8, xv8=xv8, xv8r=xvr8,
            **shared,
        ))
    return in_maps


def kernel(**inputs) -> np.ndarray:
    nc = build_program(**classify_inputs(inputs))
    in_maps = build_in_maps(inputs)
    res = run_bass_kernel_spmd(nc, in_maps, list(range(N_CORES)))
    out = np.empty((B, S, D), dtype=np.float32)
    for c in range(N_CORES):
        b = c // 4
        t0 = (c % 4) * T
        out[b, t0:t0 + T, :] = res.results[c]["out"]
    return out
